# revision 23
# baseline (speedup 1.0000x reference)
"""Trainium2 distributed kernel for channel-attention (XCA-style) module.

Reference computation (B=4, C=384, HEADS=8, HD=48, H=W=128, N=HW=16384):
  q = l2norm(in1.view(B,HEADS,HD,N), dim=-1)
  k = l2norm(in2.view(B,HEADS,HD,N), dim=-1)
  attn = softmax(q @ k^T * temperature, dim=-1)          # [B,HEADS,HD,HD]
  out  = attn @ k                                        # [B,HEADS,HD,N]
  out  = proj_w @ out + proj_b                           # 1x1 conv

Distribution: data-parallel over the spatial dim N (2048 positions/core).
Each core computes per-head partial Gram blocks for its spatial slice; ONE
AllReduce per rep (196KB) combines all batches; softmax + the fused
projection matmul are replicated; the big output matmul is local to each
core's spatial slice (host concatenates slices).

Layout: engine ops require partition bases = 0 mod 32 while head
boundaries fall at 48, so the per-head channel axes use a 64-padded
layout (8 heads x 64 rows = 4 tiles of 128; head h at tile h//2,
partitions (h%2)*64..+48). kn arrives host-padded ([B,4,112,NL], zeros in
rows 48:64) so each padded tile loads with a single DMA; SBUF padding
rows are zeroed once at kernel start.

Per-head Gram blocks are computed at PSUM partition base 0 (DoubleRow
fp8 matmuls, 2x128 spatial contraction per instruction), packed as column
blocks [0:48, h*48:(h+1)*48] of one PSUM bank.

Performance notes (from NTFF traces): per-DMA issue costs ~0.7us on the
sync engine, so DMAs are merged (whole-half-batch qkt loads, one bounce /
readback per batch, one output store per (b,ot)); reps are software
pipelined (phase A of rep r issues before phases C/D of rep r-1) so the
AllReduce latency hides behind the next rep's Gram work.
"""

import sys

import numpy as np

try:
    import concourse  # noqa: F401
except ImportError:
    sys.path.insert(0, "/opt/trn_rl_repo")

B, C, HEADS, HD = 4, 384, 8, 48
H = W = 128
N = H * W            # 16384
NCORES = 8
NL = N // NCORES     # 2048 spatial positions per core
NT2 = NL // 256      # 8 DoubleRow steps per batch
CT = C // 128        # 3 output-channel tiles
NT4 = NL // 512      # 4 output n-tiles
PT = 4               # padded d/c tiles (8 heads x 64)
TOTB = 448 * HD      # bounce elems per batch, rows 0:112 (21504)


def build_nc(nrep=1):
    import concourse.bass as bass
    import concourse.bacc as bacc
    import concourse.mybir as mybir
    from concourse.tile import TileContext

    f32 = mybir.dt.float32
    bf16 = mybir.dt.bfloat16
    fp8 = mybir.dt.float8e4
    AF = mybir.ActivationFunctionType
    DR = mybir.MatmulPerfMode.DoubleRow

    nc = bacc.Bacc()
    nc._allow_low_precision_reason = "bf16/fp8 matmul operands are intentional"

    qkt = nc.declare_dram_parameter("qkt", [B, 2, 128, 8 * 2 * C], fp8,
                                    isOutput=False)
    knp = nc.declare_dram_parameter("knp", [B, PT, 112, NL], bf16,
                                    isOutput=False)
    pwt64 = nc.declare_dram_parameter("pwt64", [PT, 128, C], bf16,
                                      isOutput=False)
    skbq = nc.declare_dram_parameter("skbq", [128, B * PT * HD], bf16,
                                     isOutput=False)
    skc64 = nc.declare_dram_parameter("skc64", [128, B * PT], f32,
                                      isOutput=False)
    biascol = nc.declare_dram_parameter("biascol", [128, CT], f32,
                                        isOutput=False)
    out = nc.declare_dram_parameter("out", [B, C, NL], bf16, isOutput=True)

    with TileContext(nc) as tc:
        with (
            tc.tile_pool(name="const", bufs=1) as cpool,
            tc.tile_pool(name="qk", bufs=5) as qkpool,
            tc.tile_pool(name="small", bufs=1) as spool,
            tc.tile_pool(name="osb", bufs=4) as opool,
            tc.tile_pool(name="psA", bufs=2, space="PSUM") as psA,
            tc.tile_pool(name="psC", bufs=2, space="PSUM") as psC,
            tc.tile_pool(name="psD", bufs=4, space="PSUM") as psD,
            tc.tile_pool(name="dram", bufs=1, space="DRAM") as dpool,
        ):
            # ---- constants ----
            pwt_sb = []
            for t in range(PT):
                p = cpool.tile([128, C], bf16, name=f"pwt{t}")
                nc.sync.dma_start(p[:, :], pwt64[t, :, :])
                pwt_sb.append(p)
            skbq_sb = cpool.tile([128, B * PT * HD], bf16)
            nc.sync.dma_start(skbq_sb[:, :], skbq[:, :])
            skc_sb = cpool.tile([128, B * PT], f32)
            nc.sync.dma_start(skc_sb[:, :], skc64[:, :])
            bias_sb = cpool.tile([128, CT], f32)
            nc.sync.dma_start(bias_sb[:, :], biascol[:, :])

            zeng = [nc.vector, nc.gpsimd]
            ei = 0

            def ecopy(eng, dst, src):
                if eng is nc.scalar:
                    nc.scalar.copy(dst, src)
                else:
                    eng.tensor_copy(dst, src)

            def escale(eng, dst, src, scale_ap):
                if eng is nc.scalar:
                    nc.scalar.mul(dst, src, scale_ap)
                else:
                    eng.tensor_scalar_mul(dst, src, scale_ap)

            # kn64[(slot, b, t)]: [128, NL] bf16; rows 0:112 DMA'd from the
            # host-padded knp (zeros already at 48:64); rows 112:128 zeroed
            # once here. Two rep-parity slots so next-rep loads never stall
            # behind this rep's phase D.
            kn_sb = {}
            for sl in range(2):
                for b in range(B):
                    for t in range(PT):
                        k = cpool.tile([128, NL], bf16, name=f"kn{sl}_{b}_{t}")
                        zeng[ei % 2].memset(k[:, :], 0.0)
                        ei += 1
                        kn_sb[(sl, b, t)] = k
            # mt64 slots: one per batch x 4 tiles, [128, C] bf16
            mt_sb = {}
            for b in range(B):
                for t in range(PT):
                    m = cpool.tile([128, C], bf16, name=f"mt{b}_{t}")
                    zeng[ei % 2].memset(m[:, :], 0.0)
                    ei += 1
                    mt_sb[(b, t)] = m
            # cgb: pre-AR compact gram per batch [128, PT*HD] bf16 (padded
            # rows zeroed once -> deterministic zeros through the AllReduce)
            cgb_sb = {}
            for b in range(B):
                g = cpool.tile([128, PT * HD], bf16, name=f"cgb{b}")
                zeng[ei % 2].memset(g[:, :], 0.0)
                ei += 1
                cgb_sb[b] = g

            cpe = [nc.scalar, nc.vector]  # PSUM-capable copy engines

            def phase_a_loads_grams(rep):
                R = str(rep)
                # all input loads first so the DMA queues saturate
                # immediately (no SP-queue stall behind extract-gated DMAs)
                sl = rep % 2
                qk_tiles = {}
                for b in range(B):
                    for half in range(2):
                        qk = qkpool.tile([128, 8 * 2 * C], fp8,
                                         name=f"qk{R}_{b}{half}", tag="qk")
                        nc.sync.dma_start(qk[:, :], qkt[b, half, :, :])
                        qk_tiles[(b, half)] = qk
                for b in range(B):
                    for t in range(PT):
                        nc.sync.dma_start(kn_sb[(sl, b, t)][0:112, :],
                                          knp[b, t, :, :])
                grams = []
                for b in range(B):
                    gram = psA.tile([128, HEADS * HD], f32, name=f"g{R}_{b}",
                                    tag="gram")
                    for half in range(2):
                        v = qk_tiles[(b, half)][:, :].rearrange(
                            "p (t c) -> p t c", t=8)
                        for i in range(4):
                            nt2 = half * 4 + i
                            first, last = nt2 == 0, nt2 == NT2 - 1
                            for h in range(HEADS):
                                nc.tensor.matmul(
                                    gram[0:HD, h * HD:(h + 1) * HD],
                                    v[:, 2 * i:2 * i + 2,
                                      h * HD:(h + 1) * HD],
                                    v[:, 2 * i:2 * i + 2,
                                      C + h * HD:C + (h + 1) * HD],
                                    start=first, stop=last, perf_mode=DR)
                    grams.append(gram)
                return grams

            def phase_a_extract_ar(rep, grams):
                R = str(rep)
                bin_a = dpool.tile([B * TOTB], bf16, name=f"bin{R}",
                                   tag="bin", bufs=2)
                bout_a0 = dpool.tile([2 * TOTB], bf16, addr_space="Shared",
                                     name=f"bout0{R}", tag="bout0", bufs=2)
                bout_a1 = dpool.tile([2 * TOTB], bf16, addr_space="Shared",
                                     name=f"bout1{R}", tag="bout1", bufs=2)
                for b in range(B):
                    gram = grams[b]
                    # PSUM -> SBUF bf16 into the padded-row cgb, one bounce
                    # DMA per batch (issued on the scalar hwdge queue)
                    if b == 2:
                        nc.gpsimd.collective_compute(
                            "AllReduce", mybir.AluOpType.add,
                            replica_groups=[list(range(NCORES))],
                            ins=[bin_a[0:2 * TOTB].opt()],
                            outs=[bout_a0[:].opt()])
                    g = cgb_sb[b]
                    for t in range(PT):
                        h0, h1 = 2 * t, 2 * t + 1
                        ecopy(cpe[t % 2], g[0:48, t * HD:(t + 1) * HD],
                              gram[0:HD, h0 * HD:(h0 + 1) * HD])
                        ecopy(cpe[(t + 1) % 2], g[64:112, t * HD:(t + 1) * HD],
                              gram[0:HD, h1 * HD:(h1 + 1) * HD])
                    nc.scalar.dma_start(
                        bin_a[b * TOTB:(b + 1) * TOTB].rearrange(
                            "(p f) -> p f", p=112),
                        g[0:112, :])
                nc.gpsimd.collective_compute(
                    "AllReduce", mybir.AluOpType.add,
                    replica_groups=[list(range(NCORES))],
                    ins=[bin_a[2 * TOTB:].opt()],
                    outs=[bout_a1[:].opt()])
                return (bout_a0, bout_a1)

            def phase_c(rep, bouts):
                R = str(rep)
                mts_all = {}
                for b in range(B):
                    gr = spool.tile([128, PT * HD], bf16, name=f"gr{R}_{b}",
                                    tag="gr", bufs=4)
                    bout_h = bouts[b // 2]
                    nc.scalar.dma_start(
                        gr[0:112, :],
                        bout_h[(b % 2) * TOTB:(b % 2 + 1) * TOTB].rearrange(
                            "(p f) -> p f", p=112))
                    # logits = gram * (s_q*temp*s_k premultiplied table)
                    l = spool.tile([128, PT * HD], f32, name=f"l{R}_{b}",
                                   tag="l", bufs=4)
                    nc.vector.tensor_mul(
                        l[:, :], gr[:, :],
                        skbq_sb[:, b * PT * HD:(b + 1) * PT * HD])
                    a = spool.tile([128, PT * HD], bf16, name=f"a{R}_{b}",
                                   tag="a", bufs=4)
                    for t in range(PT):
                        e = spool.tile([128, HD], f32, name=f"e{R}_{b}{t}",
                                       tag="e", bufs=4)
                        ssum = spool.tile([128, 1], f32, name=f"ss{R}_{b}{t}",
                                          tag="ss", bufs=4)
                        # |logits| <= max(temp): exp safe without max-sub
                        nc.scalar.activation(e[:, :],
                                             l[:, t * HD:(t + 1) * HD],
                                             AF.Exp, accum_out=ssum[:, :])
                        nc.vector.reciprocal(ssum[:, :], ssum[:, :])
                        nc.vector.tensor_scalar_mul(
                            a[:, t * HD:(t + 1) * HD], e[:, :], ssum[:, 0:1])

                    # M^T per head; fold s_k[d] in the PSUM->SBUF copy
                    mts = []
                    for t in range(PT):
                        ps = psC.tile([128, C], f32, name=f"mc{R}_{b}{t}",
                                      tag="mc")
                        for r in (0, 64):
                            rows = slice(r, r + HD)
                            nc.tensor.matmul(
                                ps[rows, :], a[rows, t * HD:(t + 1) * HD],
                                pwt_sb[t][rows, :], start=True, stop=True)
                        m = mt_sb[(b, t)]
                        col = b * PT + t
                        escale(cpe[t % 2], m[0:48, :], ps[0:48, :],
                               skc_sb[0:48, col:col + 1])
                        escale(cpe[(t + 1) % 2], m[64:112, :], ps[64:112, :],
                               skc_sb[64:112, col:col + 1])
                        mts.append(m)
                    mts_all[b] = mts

                return mts_all

            def phase_d(rep, mts_all):
                R = str(rep)
                sl = rep % 2
                ci = 0
                for b in range(B):
                    mts = mts_all[b]
                    for ot in range(CT):
                        osb = opool.tile([128, NL], bf16,
                                         name=f"os{R}_{b}{ot}", tag="osb")
                        for nt4 in range(NT4):
                            ps = psD.tile([128, 512], f32,
                                          name=f"o{R}_{b}{ot}{nt4}", tag="ops")
                            for t in range(PT):
                                nc.tensor.matmul(
                                    ps[:, :],
                                    mts[t][:, ot * 128:(ot + 1) * 128],
                                    kn_sb[(sl, b, t)][:, nt4 * 512:
                                                      (nt4 + 1) * 512],
                                    start=(t == 0), stop=(t == PT - 1))
                            dst = osb[:, nt4 * 512:(nt4 + 1) * 512]
                            eng = cpe[ci % 2]
                            ci += 1
                            if eng is nc.scalar:
                                eng.add(dst, ps[:, :], bias_sb[:, ot:ot + 1])
                            else:
                                eng.tensor_scalar_add(dst, ps[:, :],
                                                      bias_sb[:, ot:ot + 1])
                        nc.sync.dma_start(
                            out[b, ot * 128:(ot + 1) * 128, 0:NL // 2],
                            osb[:, 0:NL // 2])
                        nc.scalar.dma_start(
                            out[b, ot * 128:(ot + 1) * 128, NL // 2:NL],
                            osb[:, NL // 2:NL])

            # software-pipelined rep loop with fine-grained interleave:
            #   loads+grams(r) | readback+softmax+M+folds(r-1) |
            #   extracts+AR(r) | D+stores(r-1)
            # so the PE handoff gram(r) -> M(r-1) never waits on this rep's
            # extract copies, and the AllReduce hides behind compute.
            prev = None
            for rep in range(nrep):
                grams = phase_a_loads_grams(rep)
                if prev is not None:
                    mts_all = phase_c(prev[0], prev[1])
                bout = phase_a_extract_ar(rep, grams)
                if prev is not None:
                    phase_d(prev[0], mts_all)
                prev = (rep, bout)
            mts_all = phase_c(prev[0], prev[1])
            phase_d(prev[0], mts_all)
    nc.compile()
    return nc


def _make_in_maps(in1, in2, temperature, proj_w, proj_b):
    import ml_dtypes
    bf16 = ml_dtypes.bfloat16
    fp8 = ml_dtypes.float8_e4m3
    in1 = np.ascontiguousarray(in1, dtype=np.float32).reshape(B, C, N)
    in2 = np.ascontiguousarray(in2, dtype=np.float32).reshape(B, C, N)
    temperature = np.asarray(temperature, dtype=np.float32).reshape(HEADS)
    proj_w = np.asarray(proj_w, dtype=np.float32)
    proj_b = np.asarray(proj_b, dtype=np.float32)

    # host-side input statistics (0.9% of total FLOPs): L2 norms + scales
    EPS = 1e-12
    qn = np.maximum(np.sqrt((in1.astype(np.float64) ** 2).sum(-1)), EPS)
    kn_ = np.maximum(np.sqrt((in2.astype(np.float64) ** 2).sum(-1)), EPS)
    s_q = (1.0 / qn).astype(np.float32)        # [B, C]
    s_k = (1.0 / kn_).astype(np.float32)
    temp_c = temperature[np.arange(C) // HD]   # [C]

    # padded-64 channel mapping: tile t, partition p -> c (or -1 = padding)
    pmap = np.full((PT, 128), -1, np.int64)
    for t in range(PT):
        for r in (0, 1):
            h = 2 * t + r
            pmap[t, r * 64:r * 64 + HD] = np.arange(h * HD, (h + 1) * HD)
    valid = pmap >= 0
    cidx = np.where(valid, pmap, 0)

    skbq_t = np.zeros((128, B * PT * HD), np.float32)
    skc64 = np.zeros((128, B * PT), np.float32)
    for b in range(B):
        for t in range(PT):
            col = b * PT + t
            sq_col = np.where(valid[t], s_q[b, cidx[t]] * temp_c[cidx[t]], 0.0)
            skc64[:, col] = np.where(valid[t], s_k[b, cidx[t]], 0.0)
            head_of_p = np.where(valid[t], cidx[t] // HD, 0)
            block = s_k[b, head_of_p[:, None] * HD + np.arange(HD)[None, :]]
            skbq_t[:, col * HD:(col + 1) * HD] = sq_col[:, None] * block
    pwt64 = np.zeros((PT, 128, C), np.float32)
    for t in range(PT):
        pwt64[t][valid[t]] = proj_w[:, cidx[t][valid[t]]].T
    biascol = np.zeros((128, CT), np.float32)
    for ot in range(CT):
        biascol[:, ot] = proj_b[ot * 128:(ot + 1) * 128]

    in_maps = []
    for shard in range(NCORES):
        sl = slice(shard * NL, (shard + 1) * NL)
        qts = in1[:, :, sl].transpose(0, 2, 1)
        kts = in2[:, :, sl].transpose(0, 2, 1)
        qk_cat = np.concatenate([qts, kts], axis=-1)       # [B, NL, 2C]
        qk_tiled = qk_cat.reshape(B, 2, 8, 128, 2 * C).transpose(
            0, 1, 3, 2, 4).reshape(B, 2, 128, 8 * 2 * C)
        k_loc = in2[:, :, sl]                      # [B, C, NL]
        knp = np.zeros((B, PT, 112, NL), np.float32)
        for t in range(PT):
            knp[:, t, 0:48, :] = k_loc[:, (2 * t) * HD:(2 * t + 1) * HD, :]
            knp[:, t, 64:112, :] = k_loc[:, (2 * t + 1) * HD:
                                         (2 * t + 2) * HD, :]
        in_maps.append({
            "qkt": np.ascontiguousarray(qk_tiled).astype(fp8),
            "knp": knp.astype(bf16),
            "pwt64": pwt64.astype(bf16),
            "skbq": skbq_t.astype(bf16),
            "skc64": skc64,
            "biascol": biascol,
        })
    return in_maps


_NC_CACHE = {}


def _get_nc(nrep=1):
    if nrep not in _NC_CACHE:
        _NC_CACHE[nrep] = build_nc(nrep)
    return _NC_CACHE[nrep]


def run_cores(in_maps, trace=False):
    from concourse.bass_utils import run_bass_kernel_spmd
    nc = _get_nc()
    res = run_bass_kernel_spmd(nc, in_maps, core_ids=list(range(NCORES)),
                               trace=trace)
    return res


def kernel(in1, in2, temperature, proj_w, proj_b):
    in_maps = _make_in_maps(in1, in2, temperature, proj_w, proj_b)
    res = run_cores(in_maps, trace=False)
    full = np.empty((B, C, N), dtype=np.float32)
    for s in range(NCORES):
        full[:, :, s * NL:(s + 1) * NL] = np.asarray(
            res.results[s]["out"], dtype=np.float32)
    return full.reshape(B, C, H, W)


# revision 24
# speedup vs baseline: 1.0156x; 1.0156x over previous
"""Trainium2 distributed kernel for channel-attention (XCA-style) module.

Reference computation (B=4, C=384, HEADS=8, HD=48, H=W=128, N=HW=16384):
  q = l2norm(in1.view(B,HEADS,HD,N), dim=-1)
  k = l2norm(in2.view(B,HEADS,HD,N), dim=-1)
  attn = softmax(q @ k^T * temperature, dim=-1)          # [B,HEADS,HD,HD]
  out  = attn @ k                                        # [B,HEADS,HD,N]
  out  = proj_w @ out + proj_b                           # 1x1 conv

Distribution: data-parallel over the spatial dim N (2048 positions/core).
Each core computes per-head partial Gram blocks for its spatial slice; ONE
AllReduce per rep (196KB) combines all batches; softmax + the fused
projection matmul are replicated; the big output matmul is local to each
core's spatial slice (host concatenates slices).

Layout: engine ops require partition bases = 0 mod 32 while head
boundaries fall at 48, so the per-head channel axes use a 64-padded
layout (8 heads x 64 rows = 4 tiles of 128; head h at tile h//2,
partitions (h%2)*64..+48). kn arrives host-padded ([B,4,112,NL], zeros in
rows 48:64) so each padded tile loads with a single DMA; SBUF padding
rows are zeroed once at kernel start.

Per-head Gram blocks are computed at PSUM partition base 0 (DoubleRow
fp8 matmuls, 2x128 spatial contraction per instruction), packed as column
blocks [0:48, h*48:(h+1)*48] of one PSUM bank.

Performance notes (from NTFF traces): per-DMA issue costs ~0.7us on the
sync engine, so DMAs are merged (whole-half-batch qkt loads, one bounce /
readback per batch, one output store per (b,ot)); reps are software
pipelined (phase A of rep r issues before phases C/D of rep r-1) so the
AllReduce latency hides behind the next rep's Gram work.
"""

import sys

import numpy as np

try:
    import concourse  # noqa: F401
except ImportError:
    sys.path.insert(0, "/opt/trn_rl_repo")

B, C, HEADS, HD = 4, 384, 8, 48
H = W = 128
N = H * W            # 16384
NCORES = 8
NL = N // NCORES     # 2048 spatial positions per core
NT2 = NL // 256      # 8 DoubleRow steps per batch
CT = C // 128        # 3 output-channel tiles
NT4 = NL // 512      # 4 output n-tiles
PT = 4               # padded d/c tiles (8 heads x 64)
TOTB = 448 * HD      # bounce elems per batch, rows 0:112 (21504)


def build_nc(nrep=1):
    import concourse.bass as bass
    import concourse.bacc as bacc
    import concourse.mybir as mybir
    from concourse.tile import TileContext

    f32 = mybir.dt.float32
    bf16 = mybir.dt.bfloat16
    fp8 = mybir.dt.float8e4
    AF = mybir.ActivationFunctionType
    DR = mybir.MatmulPerfMode.DoubleRow

    nc = bacc.Bacc()
    nc._allow_low_precision_reason = "bf16/fp8 matmul operands are intentional"

    qkt = nc.declare_dram_parameter("qkt", [B, 2, 128, 8 * 2 * C], fp8,
                                    isOutput=False)
    knp = nc.declare_dram_parameter("knp", [B, PT, 112, NL], bf16,
                                    isOutput=False)
    pwt64 = nc.declare_dram_parameter("pwt64", [PT, 128, C], bf16,
                                      isOutput=False)
    skbq = nc.declare_dram_parameter("skbq", [128, B * PT * HD], bf16,
                                     isOutput=False)
    skc64 = nc.declare_dram_parameter("skc64", [128, B * PT], f32,
                                      isOutput=False)
    biascol = nc.declare_dram_parameter("biascol", [128, CT], f32,
                                        isOutput=False)
    out = nc.declare_dram_parameter("out", [B, C, NL], bf16, isOutput=True)

    with TileContext(nc) as tc:
        with (
            tc.tile_pool(name="const", bufs=1) as cpool,
            tc.tile_pool(name="qk", bufs=5) as qkpool,
            tc.tile_pool(name="small", bufs=1) as spool,
            tc.tile_pool(name="osb", bufs=4) as opool,
            tc.tile_pool(name="psA", bufs=2, space="PSUM") as psA,
            tc.tile_pool(name="psC", bufs=2, space="PSUM") as psC,
            tc.tile_pool(name="psD", bufs=4, space="PSUM") as psD,
            tc.tile_pool(name="dram", bufs=1, space="DRAM") as dpool,
        ):
            # ---- constants ----
            pwt_sb = []
            for t in range(PT):
                p = cpool.tile([128, C], bf16, name=f"pwt{t}")
                nc.sync.dma_start(p[:, :], pwt64[t, :, :])
                pwt_sb.append(p)
            skbq_sb = cpool.tile([128, B * PT * HD], bf16)
            nc.sync.dma_start(skbq_sb[:, :], skbq[:, :])
            skc_sb = cpool.tile([128, B * PT], f32)
            nc.sync.dma_start(skc_sb[:, :], skc64[:, :])
            bias_sb = cpool.tile([128, CT], f32)
            nc.sync.dma_start(bias_sb[:, :], biascol[:, :])

            zeng = [nc.vector, nc.gpsimd]
            ei = 0

            def ecopy(eng, dst, src):
                if eng is nc.scalar:
                    nc.scalar.copy(dst, src)
                else:
                    eng.tensor_copy(dst, src)

            def escale(eng, dst, src, scale_ap):
                if eng is nc.scalar:
                    nc.scalar.mul(dst, src, scale_ap)
                else:
                    eng.tensor_scalar_mul(dst, src, scale_ap)

            # kn64[(slot, b, t)]: [128, NL] bf16; rows 0:112 DMA'd from the
            # host-padded knp (zeros already at 48:64); rows 112:128 zeroed
            # once here. Two rep-parity slots so next-rep loads never stall
            # behind this rep's phase D.
            kn_sb = {}
            for sl in range(2):
                for b in range(B):
                    for t in range(PT):
                        k = cpool.tile([128, NL], bf16, name=f"kn{sl}_{b}_{t}")
                        zeng[ei % 2].memset(k[:, :], 0.0)
                        ei += 1
                        kn_sb[(sl, b, t)] = k
            # mt64 slots: one per batch x 4 tiles, [128, C] bf16
            mt_sb = {}
            for b in range(B):
                for t in range(PT):
                    m = cpool.tile([128, C], bf16, name=f"mt{b}_{t}")
                    zeng[ei % 2].memset(m[:, :], 0.0)
                    ei += 1
                    mt_sb[(b, t)] = m
            # cgb: pre-AR compact gram per batch [128, PT*HD] bf16 (padded
            # rows zeroed once -> deterministic zeros through the AllReduce)
            cgb_sb = {}
            for b in range(B):
                g = cpool.tile([128, PT * HD], bf16, name=f"cgb{b}")
                zeng[ei % 2].memset(g[:, :], 0.0)
                ei += 1
                cgb_sb[b] = g

            cpe = [nc.scalar, nc.vector]  # PSUM-capable copy engines

            def phase_a_loads_grams(rep):
                R = str(rep)
                # all input loads first so the DMA queues saturate
                # immediately (no SP-queue stall behind extract-gated DMAs)
                sl = rep % 2
                qk_tiles = {}
                for b in range(B):
                    for half in range(2):
                        qk = qkpool.tile([128, 8 * 2 * C], fp8,
                                         name=f"qk{R}_{b}{half}", tag="qk")
                        nc.sync.dma_start(qk[:, :], qkt[b, half, :, :])
                        qk_tiles[(b, half)] = qk
                for b in range(B):
                    for t in range(PT):
                        nc.sync.dma_start(kn_sb[(sl, b, t)][0:112, :],
                                          knp[b, t, :, :])
                grams = []
                for b in range(B):
                    gram = psA.tile([128, HEADS * HD], f32, name=f"g{R}_{b}",
                                    tag="gram")
                    for half in range(2):
                        v = qk_tiles[(b, half)][:, :].rearrange(
                            "p (t c) -> p t c", t=8)
                        for i in range(4):
                            nt2 = half * 4 + i
                            first, last = nt2 == 0, nt2 == NT2 - 1
                            for h in range(HEADS):
                                nc.tensor.matmul(
                                    gram[0:HD, h * HD:(h + 1) * HD],
                                    v[:, 2 * i:2 * i + 2,
                                      h * HD:(h + 1) * HD],
                                    v[:, 2 * i:2 * i + 2,
                                      C + h * HD:C + (h + 1) * HD],
                                    start=first, stop=last, perf_mode=DR)
                    grams.append(gram)
                return grams

            def phase_a_extract_ar(rep, grams):
                R = str(rep)
                bin_a = dpool.tile([B * TOTB], bf16, name=f"bin{R}",
                                   tag="bin", bufs=2)
                bout_a = dpool.tile([B * TOTB], bf16, addr_space="Shared",
                                    name=f"bout{R}", tag="bout", bufs=2)
                for b in range(B):
                    gram = grams[b]
                    # PSUM -> SBUF bf16 into the padded-row cgb, one bounce
                    # DMA per batch (issued on the scalar hwdge queue)
                    g = cgb_sb[b]
                    for t in range(PT):
                        h0, h1 = 2 * t, 2 * t + 1
                        ecopy(cpe[t % 2], g[0:48, t * HD:(t + 1) * HD],
                              gram[0:HD, h0 * HD:(h0 + 1) * HD])
                        ecopy(cpe[(t + 1) % 2], g[64:112, t * HD:(t + 1) * HD],
                              gram[0:HD, h1 * HD:(h1 + 1) * HD])
                    nc.scalar.dma_start(
                        bin_a[b * TOTB:(b + 1) * TOTB].rearrange(
                            "(p f) -> p f", p=112),
                        g[0:112, :])
                nc.gpsimd.collective_compute(
                    "AllReduce", mybir.AluOpType.add,
                    replica_groups=[list(range(NCORES))],
                    ins=[bin_a[:].opt()], outs=[bout_a[:].opt()])
                return bout_a

            def phase_c(rep, bout_a):
                R = str(rep)
                mts_all = {}
                for b in range(B):
                    gr = spool.tile([128, PT * HD], bf16, name=f"gr{R}_{b}",
                                    tag="gr", bufs=4)
                    nc.scalar.dma_start(
                        gr[0:112, :],
                        bout_a[b * TOTB:(b + 1) * TOTB].rearrange(
                            "(p f) -> p f", p=112))
                    # logits = gram * (s_q*temp*s_k premultiplied table)
                    l = spool.tile([128, PT * HD], f32, name=f"l{R}_{b}",
                                   tag="l", bufs=4)
                    nc.vector.tensor_mul(
                        l[:, :], gr[:, :],
                        skbq_sb[:, b * PT * HD:(b + 1) * PT * HD])
                    a = spool.tile([128, PT * HD], bf16, name=f"a{R}_{b}",
                                   tag="a", bufs=4)
                    for t in range(PT):
                        e = spool.tile([128, HD], f32, name=f"e{R}_{b}{t}",
                                       tag="e", bufs=4)
                        ssum = spool.tile([128, 1], f32, name=f"ss{R}_{b}{t}",
                                          tag="ss", bufs=4)
                        # |logits| <= max(temp): exp safe without max-sub
                        nc.scalar.activation(e[:, :],
                                             l[:, t * HD:(t + 1) * HD],
                                             AF.Exp, accum_out=ssum[:, :])
                        nc.vector.reciprocal(ssum[:, :], ssum[:, :])
                        nc.vector.tensor_scalar_mul(
                            a[:, t * HD:(t + 1) * HD], e[:, :], ssum[:, 0:1])

                    # M^T per head; fold s_k[d] in the PSUM->SBUF copy
                    mts = []
                    for t in range(PT):
                        ps = psC.tile([128, C], f32, name=f"mc{R}_{b}{t}",
                                      tag="mc")
                        for r in (0, 64):
                            rows = slice(r, r + HD)
                            nc.tensor.matmul(
                                ps[rows, :], a[rows, t * HD:(t + 1) * HD],
                                pwt_sb[t][rows, :], start=True, stop=True)
                        m = mt_sb[(b, t)]
                        col = b * PT + t
                        escale(cpe[t % 2], m[0:48, :], ps[0:48, :],
                               skc_sb[0:48, col:col + 1])
                        escale(cpe[(t + 1) % 2], m[64:112, :], ps[64:112, :],
                               skc_sb[64:112, col:col + 1])
                        mts.append(m)
                    mts_all[b] = mts

                return mts_all

            def phase_d(rep, mts_all):
                R = str(rep)
                sl = rep % 2
                ci = 0
                for b in range(B):
                    mts = mts_all[b]
                    for ot in range(CT):
                        osb = opool.tile([128, NL], bf16,
                                         name=f"os{R}_{b}{ot}", tag="osb")
                        for nt4 in range(NT4):
                            ps = psD.tile([128, 512], f32,
                                          name=f"o{R}_{b}{ot}{nt4}", tag="ops")
                            for t in range(PT):
                                nc.tensor.matmul(
                                    ps[:, :],
                                    mts[t][:, ot * 128:(ot + 1) * 128],
                                    kn_sb[(sl, b, t)][:, nt4 * 512:
                                                      (nt4 + 1) * 512],
                                    start=(t == 0), stop=(t == PT - 1))
                            dst = osb[:, nt4 * 512:(nt4 + 1) * 512]
                            eng = cpe[ci % 2]
                            ci += 1
                            if eng is nc.scalar:
                                eng.add(dst, ps[:, :], bias_sb[:, ot:ot + 1])
                            else:
                                eng.tensor_scalar_add(dst, ps[:, :],
                                                      bias_sb[:, ot:ot + 1])
                        nc.sync.dma_start(
                            out[b, ot * 128:(ot + 1) * 128, 0:NL // 2],
                            osb[:, 0:NL // 2])
                        nc.scalar.dma_start(
                            out[b, ot * 128:(ot + 1) * 128, NL // 2:NL],
                            osb[:, NL // 2:NL])

            # software-pipelined rep loop with fine-grained interleave:
            #   loads+grams(r) | readback+softmax+M+folds(r-1) |
            #   extracts+AR(r) | D+stores(r-1)
            # so the PE handoff gram(r) -> M(r-1) never waits on this rep's
            # extract copies, and the AllReduce hides behind compute.
            prev = None
            for rep in range(nrep):
                grams = phase_a_loads_grams(rep)
                if prev is not None:
                    mts_all = phase_c(prev[0], prev[1])
                bout = phase_a_extract_ar(rep, grams)
                if prev is not None:
                    phase_d(prev[0], mts_all)
                prev = (rep, bout)
            mts_all = phase_c(prev[0], prev[1])
            phase_d(prev[0], mts_all)
    nc.compile()
    return nc


def _make_in_maps(in1, in2, temperature, proj_w, proj_b):
    import ml_dtypes
    bf16 = ml_dtypes.bfloat16
    fp8 = ml_dtypes.float8_e4m3
    in1 = np.ascontiguousarray(in1, dtype=np.float32).reshape(B, C, N)
    in2 = np.ascontiguousarray(in2, dtype=np.float32).reshape(B, C, N)
    temperature = np.asarray(temperature, dtype=np.float32).reshape(HEADS)
    proj_w = np.asarray(proj_w, dtype=np.float32)
    proj_b = np.asarray(proj_b, dtype=np.float32)

    # host-side input statistics (0.9% of total FLOPs): L2 norms + scales
    EPS = 1e-12
    qn = np.maximum(np.sqrt((in1.astype(np.float64) ** 2).sum(-1)), EPS)
    kn_ = np.maximum(np.sqrt((in2.astype(np.float64) ** 2).sum(-1)), EPS)
    s_q = (1.0 / qn).astype(np.float32)        # [B, C]
    s_k = (1.0 / kn_).astype(np.float32)
    temp_c = temperature[np.arange(C) // HD]   # [C]

    # padded-64 channel mapping: tile t, partition p -> c (or -1 = padding)
    pmap = np.full((PT, 128), -1, np.int64)
    for t in range(PT):
        for r in (0, 1):
            h = 2 * t + r
            pmap[t, r * 64:r * 64 + HD] = np.arange(h * HD, (h + 1) * HD)
    valid = pmap >= 0
    cidx = np.where(valid, pmap, 0)

    skbq_t = np.zeros((128, B * PT * HD), np.float32)
    skc64 = np.zeros((128, B * PT), np.float32)
    for b in range(B):
        for t in range(PT):
            col = b * PT + t
            sq_col = np.where(valid[t], s_q[b, cidx[t]] * temp_c[cidx[t]], 0.0)
            skc64[:, col] = np.where(valid[t], s_k[b, cidx[t]], 0.0)
            head_of_p = np.where(valid[t], cidx[t] // HD, 0)
            block = s_k[b, head_of_p[:, None] * HD + np.arange(HD)[None, :]]
            skbq_t[:, col * HD:(col + 1) * HD] = sq_col[:, None] * block
    pwt64 = np.zeros((PT, 128, C), np.float32)
    for t in range(PT):
        pwt64[t][valid[t]] = proj_w[:, cidx[t][valid[t]]].T
    biascol = np.zeros((128, CT), np.float32)
    for ot in range(CT):
        biascol[:, ot] = proj_b[ot * 128:(ot + 1) * 128]

    in_maps = []
    for shard in range(NCORES):
        sl = slice(shard * NL, (shard + 1) * NL)
        qts = in1[:, :, sl].transpose(0, 2, 1)
        kts = in2[:, :, sl].transpose(0, 2, 1)
        qk_cat = np.concatenate([qts, kts], axis=-1)       # [B, NL, 2C]
        qk_tiled = qk_cat.reshape(B, 2, 8, 128, 2 * C).transpose(
            0, 1, 3, 2, 4).reshape(B, 2, 128, 8 * 2 * C)
        k_loc = in2[:, :, sl]                      # [B, C, NL]
        knp = np.zeros((B, PT, 112, NL), np.float32)
        for t in range(PT):
            knp[:, t, 0:48, :] = k_loc[:, (2 * t) * HD:(2 * t + 1) * HD, :]
            knp[:, t, 64:112, :] = k_loc[:, (2 * t + 1) * HD:
                                         (2 * t + 2) * HD, :]
        in_maps.append({
            "qkt": np.ascontiguousarray(qk_tiled).astype(fp8),
            "knp": knp.astype(bf16),
            "pwt64": pwt64.astype(bf16),
            "skbq": skbq_t.astype(bf16),
            "skc64": skc64,
            "biascol": biascol,
        })
    return in_maps


_NC_CACHE = {}


def _get_nc(nrep=1):
    if nrep not in _NC_CACHE:
        _NC_CACHE[nrep] = build_nc(nrep)
    return _NC_CACHE[nrep]


def run_cores(in_maps, trace=False):
    from concourse.bass_utils import run_bass_kernel_spmd
    nc = _get_nc()
    res = run_bass_kernel_spmd(nc, in_maps, core_ids=list(range(NCORES)),
                               trace=trace)
    return res


def kernel(in1, in2, temperature, proj_w, proj_b):
    in_maps = _make_in_maps(in1, in2, temperature, proj_w, proj_b)
    res = run_cores(in_maps, trace=False)
    full = np.empty((B, C, N), dtype=np.float32)
    for s in range(NCORES):
        full[:, :, s * NL:(s + 1) * NL] = np.asarray(
            res.results[s]["out"], dtype=np.float32)
    return full.reshape(B, C, H, W)
